# revision 33
# baseline (speedup 1.0000x reference)
"""Trainium2 Bass kernel for nn_AttentionModel (B=4, S=4096, E=2048) on 8 cores.

Gram-matrix restructuring: since q = xWq^T + bq and k = xWk^T + bk,
    scores*sqrt(E) = Wq (x^T x) Wk^T + bq(Wk xs + S bk)^T + (Wq xs) bk^T
with xs = column-sums of x (rank-1 terms host-precomputed), and
    out = attn v = (attn Wv) x^T + (attn bv) 1^T.
This cuts total FLOPs from 687 GF to 481 GF and removes the explicit
q/k/v projections entirely.

Sharding: one batch per pair of cores; within a pair, core h owns e-rows
[h*1024,(h+1)*1024) of scores/out. Per core:
  A: Ghat = x^T x[:, own-half]   [2048, 1024]  (17.2 GF)
  B: T_h  = Ghat^T-contract Wk^T [1024, 2048]  ( 8.6 GF)  -> pairwise
     AllGather of T halves, pipelined in 4 f-chunks of 512
  C: scores_h = WqT_h^T T (+rank-2 bias)       ( 8.6 GF), softmax
  D: P^T = Wv^T-contract attn^T  [2048, 1024]  ( 8.6 GF, bf16)
  E: out_h = P^T^T x^T (+bv rank-1) [1024, 4096] (17.2 GF, bf16)
Total 60.2 GF/core vs 120.8 GF/core for the direct data-parallel kernel.

x columns (and Wk^T rows) are host-permuted so each core's own e-half is
first; T rows land in natural global order after the AllGather, so the
scores contraction uses unpermuted WqT_h. The scores path stays f32r;
attn/P/x^T in the output path are bf16 (error << the 2e-2 gate).
"""

import sys

sys.path.insert(0, "/opt/trn_rl_repo")

from contextlib import ExitStack

import numpy as np

import concourse.bass as bass
import concourse.mybir as mybir
import concourse.tile as tile
from concourse import bacc
from concourse.bass_utils import run_bass_kernel_spmd
from concourse.masks import make_identity

f32 = mybir.dt.float32
f32r = mybir.dt.float32r
bf16 = mybir.dt.bfloat16
f16 = mybir.dt.float16

B, S, E = 4, 4096, 2048
EH = E // 2          # per-core e rows
FC = 512             # CC f-chunk width
NFC = E // FC        # 4 chunks
JC = E // 128        # 16 contraction chunks of 128
ET = EH // 128       # 8 e'-tiles
SB = 1024            # out s-block
N_CORES = 8
PAIRS = [[0, 1], [2, 3], [4, 5], [6, 7]]


def build_kernel():
    nc = bacc.Bacc("TRN2", debug=False, target_bir_lowering=False, num_devices=8)

    x_nat = nc.dram_tensor("x_nat", [S, E], f16, kind="ExternalInput")
    wkt = nc.dram_tensor("wkt", [NFC // 2, 128, JC, 2 * FC], f16, kind="ExternalInput")
    wqlo_d = nc.dram_tensor("wqlo", [128, JC, EH // 2], f16, kind="ExternalInput")
    wqhi_d = nc.dram_tensor("wqhi", [128, JC, EH // 2], f16, kind="ExternalInput")
    bias_lhs = nc.dram_tensor("bias_lhs", [2, EH], f16, kind="ExternalInput")
    bias_rhs = nc.dram_tensor("bias_rhs", [2, E], f16, kind="ExternalInput")
    wv = nc.dram_tensor("wv", [JC, 128, JC, 128], bf16, kind="ExternalInput")
    onesbv_d = nc.dram_tensor("onesbv", [128, JC, 2], bf16, kind="ExternalInput")
    xt = nc.dram_tensor("xt", [4, 128, JC, SB], bf16, kind="ExternalInput")
    outt = nc.dram_tensor("outt", [EH, S], f32, kind="ExternalOutput")

    with tile.TileContext(nc) as tc, ExitStack() as ctx:
        dram = ctx.enter_context(tc.tile_pool(name="dram", bufs=1, space="DRAM"))
        ccin = [dram.tile([EH // 2, 2 * FC], f16, name=f"ccin{i}") for i in range(4)]
        ccout = [
            dram.tile([2, EH // 2, 2 * FC], f16, name=f"ccout{i}")
            for i in range(4)
        ]

        const = ctx.enter_context(tc.tile_pool(name="const", bufs=1))
        ident = const.tile([128, 128], f16)
        make_identity(nc, ident[:, :])
        onesbv_sb = const.tile([128, JC, 2], bf16)
        nc.gpsimd.dma_start(onesbv_sb[:, :, :], onesbv_d[:, :, :])
        shift_sb = const.tile([128, 1], f32)
        nc.gpsimd.memset(shift_sb[:, :], -20.0)

        # 1/rowsum and bv-bias per e'-row, [128, ET] layouts; live C'..E
        rs_pool = ctx.enter_context(tc.tile_pool(name="rs", bufs=1, side="right"))
        rp_col = rs_pool.tile([128, 2, ET], f32)
        rcol = rs_pool.tile([128, ET], f32)
        pbvscol = rs_pool.tile([128, ET], f32)

        # wq halves (moving side of scoresT GEMM) load during A
        wqlo_es = ExitStack()
        wqlo_pool = wqlo_es.enter_context(tc.tile_pool(name="wqlo", bufs=1))
        wq_lo = wqlo_pool.tile([128, JC, EH // 2], f16)
        wqhi_es = ExitStack()
        wqhi_pool = wqhi_es.enter_context(tc.tile_pool(name="wqhi", bufs=1))
        wq_hi = wqhi_pool.tile([128, JC, EH // 2], f16)

        # ---- Phase A: Ghat = x^T x[:, own-half], fp16 accumulate ----
        with tc.tile_pool(name="gsb16", bufs=1) as g16pool:
            gsb16 = g16pool.tile([128, JC, EH], f16)
            with tc.tile_pool(name="wk", bufs=2) as wkpool:
                wk_first = {}
                with (
                    tc.tile_pool(name="xg", bufs=2) as xpool,
                    tc.tile_pool(name="psA", bufs=2, space="PSUM") as psA,
                ):
                    for g in range(8):  # s-groups of 4x128 rows
                        xg = xpool.tile([128, 4, E], f16, tag="xg")
                        engs = [nc.scalar, nc.sync, nc.gpsimd, nc.scalar]
                        for c in range(4):
                            eng = engs[c] if g == 0 else (
                                nc.scalar if c % 2 == 0 else nc.sync)
                            eng.dma_start(
                                xg[:, c, :],
                                x_nat[g * 512 + c * 128:
                                      g * 512 + (c + 1) * 128, :],
                            )
                        if g == 2:
                            # wq loads deferred past startup
                            nc.gpsimd.dma_start(wq_lo[:, :, :], wqlo_d[:, :, :])
                            nc.gpsimd.dma_start(wq_hi[:, :, :], wqhi_d[:, :, :])
                        if g == 5:
                            # preload first Wk pair while A finishes
                            wk0 = wkpool.tile([128, JC, 2 * FC], f16, tag="wk")
                            nc.gpsimd.dma_start(wk0[:, :, :], wkt[0])
                            wk_first[0] = wk0
                        for it in range(JC):
                            # G[own,own] is symmetric: for a-chunks 4..7 the
                            # b<512 half lies below the diagonal block row --
                            # filled later by transposing the mirror tiles
                            u0 = 1 if 4 <= it < 8 else 0
                            lo = u0 * 512
                            ps = psA.tile([128, EH], f32, tag="psA")
                            for c in range(4):
                                lhsT = xg[:, c, it * 128:(it + 1) * 128]
                                for u in range(u0, 2):
                                    nc.tensor.matmul(
                                        ps[:, u * 512:(u + 1) * 512],
                                        lhsT,
                                        xg[:, c, u * 512:(u + 1) * 512],
                                        start=(c == 0),
                                        stop=(c == 3),
                                    )
                            if g == 0:
                                nc.vector.tensor_copy(
                                    gsb16[:, it, lo:], ps[:, lo:]
                                )
                            else:
                                nc.vector.tensor_add(
                                    gsb16[:, it, lo:],
                                    gsb16[:, it, lo:], ps[:, lo:],
                                )

                # fill Ghat[4..8)[:, 0:512] = transpose of mirror tiles
                with tc.tile_pool(name="psF", bufs=2, space="PSUM") as psF:
                    for it in range(4, 8):
                        pf = psF.tile([128, 512], f16, tag="psF")
                        for bc in range(4):
                            nc.tensor.transpose(
                                pf[:, bc * 128:(bc + 1) * 128],
                                gsb16[:, bc, it * 128:(it + 1) * 128],
                                ident[:, :],
                            )
                        nc.vector.tensor_copy(gsb16[:, it, 0:512], pf[:, :])

                # -- Phase B: T_h = Ghat^T-contract WkT (fp16); AllGather --
                with (
                    tc.tile_pool(name="stB", bufs=3) as stB,
                    tc.tile_pool(name="psB", bufs=3, space="PSUM") as psB,
                ):
                    for pc in range(NFC // 2):
                        if pc in wk_first:
                            wk_sb = wk_first[pc]
                        else:
                            wk_sb = wkpool.tile(
                                [128, JC, 2 * FC], f16, tag="wk"
                            )
                            nc.sync.dma_start(wk_sb[:, :, :], wkt[pc])
                        for mt in range(ET):
                            ps = psB.tile([128, 2 * FC], f32, tag="psB")
                            for ac in range(JC):
                                for u in range(2):
                                    nc.tensor.matmul(
                                        ps[:, u * FC:(u + 1) * FC],
                                        gsb16[:, ac, mt * 128:(mt + 1) * 128],
                                        wk_sb[:, ac, u * FC:(u + 1) * FC],
                                        start=(ac == 0),
                                        stop=(ac == JC - 1),
                                    )
                            st = stB.tile([128, 2 * FC], f16, tag="stB")
                            nc.scalar.copy(st[:, :], ps[:, :])
                            piece = 2 * pc + mt // 4
                            nc.gpsimd.dma_start(
                                ccin[piece][(mt % 4) * 128:
                                            (mt % 4 + 1) * 128, :],
                                st[:, :],
                            )
                            if mt % 4 == 3:
                                nc.gpsimd.collective_compute(
                                    "AllGather",
                                    mybir.AluOpType.bypass,
                                    replica_groups=PAIRS,
                                    ins=[ccin[piece][:, :]],
                                    outs=[ccout[piece][:, :, :]],
                                )

        # ---- Phase C': scoresT = T^T-contract Wq + bias^T, exp'd in place;
        #      row-sums and bv-bias via [ones|bv] rank-2 over bf16 attnT ----
        wv_es = ExitStack()
        wvpool = wv_es.enter_context(
            tc.tile_pool(name="wv", bufs=3, side="right")
        )
        wv_first = {}
        atT_es = ExitStack()
        atT_pool = atT_es.enter_context(
            tc.tile_pool(name="atT", bufs=1, side="right")
        )
        attnT = atT_pool.tile([128, JC, EH], bf16)
        with (
            tc.tile_pool(name="cb", bufs=1) as cbpool,
            tc.tile_pool(name="tfc", bufs=2) as tpool,
            tc.tile_pool(name="psC", bufs=2, space="PSUM") as psC,
            tc.tile_pool(name="psS", bufs=1, space="PSUM") as psS,
        ):
            bl_sb = cbpool.tile([2, EH], f16)
            nc.gpsimd.dma_start(bl_sb[:, :], bias_lhs[:, :])
            br_sb = cbpool.tile([2, E], f16)
            nc.gpsimd.dma_start(br_sb[:, :], bias_rhs[:, :])
            sums_ps = psS.tile([2, EH], f32)
            for pc in range(NFC // 2):
                # T chunk split by pair-slab across two DMA queues;
                # each slab half arrives as two row pieces
                tlo = tpool.tile([128, ET, 2 * FC], f16, tag="tlo")
                thi = tpool.tile([128, ET, 2 * FC], f16, tag="thi")
                for half in range(2):
                    piece = 2 * pc + half
                    nc.scalar.dma_start(
                        tlo[:, half * 4:(half + 1) * 4, :],
                        ccout[piece][0].rearrange("(r p) f -> p r f", p=128),
                    )
                    nc.sync.dma_start(
                        thi[:, half * 4:(half + 1) * 4, :],
                        ccout[piece][1].rearrange("(r p) f -> p r f", p=128),
                    )
                for ftile in range(2 * FC // 128):
                    fkt = pc * (2 * FC // 128) + ftile
                    ps = psC.tile([128, EH], f32, tag="psC")
                    ic_order = [0, 1, 2, 3, 8, 9, 10, 11,
                                4, 5, 6, 7, 12, 13, 14, 15]
                    for k, ic in enumerate(ic_order):
                        tsrc = tlo if ic < ET else thi
                        lhsT = tsrc[:, ic % ET, ftile * 128:(ftile + 1) * 128]
                        for u in range(2):
                            mov = wq_lo if u == 0 else wq_hi
                            nc.tensor.matmul(
                                ps[:, u * 512:(u + 1) * 512],
                                lhsT,
                                mov[:, ic, :],
                                start=(k == 0),
                                stop=False,
                            )
                    for u in range(2):
                        nc.tensor.matmul(
                            ps[:, u * 512:(u + 1) * 512],
                            br_sb[:, fkt * 128:(fkt + 1) * 128],
                            bl_sb[:, u * 512:(u + 1) * 512],
                            start=False,
                            stop=True,
                        )
                    # global shift keeps exp in the act-table domain;
                    # softmax is invariant to a constant shift (scores<~15)
                    nc.scalar.activation(
                        attnT[:, fkt, :], ps[:, :],
                        mybir.ActivationFunctionType.Exp,
                        bias=shift_sb[:, 0:1], scale=1.0 / 64.0,
                    )
            # prefetch first Wv tiles for phase D during the C' tail
            for jt in range(2):
                wvt = wvpool.tile([128, JC, 128], bf16, tag="wv")
                nc.gpsimd.dma_start(wvt[:, :, :], wv[jt])
                wv_first[jt] = wvt
            # row-sums + bv-bias in one bf16 block (avoids PE dtype
            # mode switches inside the scores stream)
            for fkt in range(JC):
                for u in range(2):
                    nc.tensor.matmul(
                        sums_ps[:, u * 512:(u + 1) * 512],
                        onesbv_sb[:, fkt, :],
                        attnT[:, fkt, u * 512:(u + 1) * 512],
                        start=(fkt == 0),
                        stop=(fkt == JC - 1),
                    )
            # [sums; pbv] [2, EH] -> per-partition [128, 2, ET] layout,
            # via a DRAM bounce (SBUF APs can't scatter free dim->partition)
            rp_row = cbpool.tile([2, EH], f32)
            nc.scalar.copy(rp_row[:, :], sums_ps[:, :])
            rp_d = dram.tile([2, EH], f32)
            nc.sync.dma_start(rp_d[:, :], rp_row[:, :])
            nc.sync.dma_start(
                rp_col[:, :, :],
                rp_d[:, :].rearrange("r (et p) -> p r et", p=128),
            )
            nc.vector.reciprocal(rcol[:, :], rp_col[:, 0, :])
            nc.vector.tensor_mul(pbvscol[:, :], rp_col[:, 1, :], rcol[:, :])
        wqhi_es.close()
        wqlo_es.close()

        # ---- Phase D: P^T = Wv^T-contract attnT ----
        pt_pool = ctx.enter_context(tc.tile_pool(name="pt", bufs=1))
        pt_sb = pt_pool.tile([128, JC, EH], bf16)
        xt_es = ExitStack()
        xtpool = xt_es.enter_context(tc.tile_pool(name="xtq", bufs=2))
        xq_first = {}
        with (
            tc.tile_pool(name="psD", bufs=2, space="PSUM") as psD,
        ):
            # prefetch first x^T quarter for phase E
            xq0 = xtpool.tile([128, JC, SB], bf16, tag="xq")
            nc.gpsimd.dma_start(xq0[:, :, :], xt[0])
            xq_first[0] = xq0
            for jt in range(JC):
                if jt in wv_first:
                    wv_sb = wv_first[jt]
                else:
                    wv_sb = wvpool.tile([128, JC, 128], bf16, tag="wv")
                    nc.sync.dma_start(wv_sb[:, :, :], wv[jt])
                ps = psD.tile([128, EH], f32, tag="psD")
                for fkt in range(JC):
                    for u in range(2):
                        nc.tensor.matmul(
                            ps[:, u * 512:(u + 1) * 512],
                            wv_sb[:, fkt, :],
                            attnT[:, fkt, u * 512:(u + 1) * 512],
                            start=(fkt == 0),
                            stop=(fkt == JC - 1),
                        )
                nc.scalar.copy(pt_sb[:, jt, :], ps[:, :])
        atT_es.close()
        wv_es.close()

        # ---- Phase E: out_h = (P x^T) * rsum + pbv*rsum (fused eviction) ----
        with (
            tc.tile_pool(name="stE", bufs=3) as stE,
            tc.tile_pool(name="psE", bufs=2, space="PSUM") as psE,
        ):
            for sb in range(S // SB):
                if sb in xq_first:
                    xq = xq_first[sb]
                else:
                    xq = xtpool.tile([128, JC, SB], bf16, tag="xq")
                    nc.gpsimd.dma_start(xq[:, :, :], xt[sb])
                for et in range(ET):
                    ps = psE.tile([128, SB], f32, tag="psE")
                    for jc in range(JC):
                        for u in range(2):
                            nc.tensor.matmul(
                                ps[:, u * 512:(u + 1) * 512],
                                pt_sb[:, jc, et * 128:(et + 1) * 128],
                                xq[:, jc, u * 512:(u + 1) * 512],
                                start=(jc == 0),
                                stop=(jc == JC - 1),
                            )
                    ost = stE.tile([128, SB], f32, tag="stE")
                    nc.vector.tensor_scalar(
                        out=ost[:, :], in0=ps[:, :],
                        scalar1=rcol[:, et:et + 1],
                        scalar2=pbvscol[:, et:et + 1],
                        op0=mybir.AluOpType.mult,
                        op1=mybir.AluOpType.add,
                    )
                    nc.sync.dma_start(
                        outt[et * 128:(et + 1) * 128, sb * SB:(sb + 1) * SB],
                        ost[:, :],
                    )
        xt_es.close()

    nc.compile()
    return nc


_NC_CACHE = {}


def _get_nc():
    if "nc" not in _NC_CACHE:
        _NC_CACHE["nc"] = build_kernel()
    return _NC_CACHE["nc"]


def make_in_maps(x, Wq, bq, Wk, bk, Wv, bv):
    import ml_dtypes

    bft = ml_dtypes.bfloat16
    sc = np.float32(1.0 / np.sqrt(E))
    x = np.asarray(x, np.float32)
    Wq = np.asarray(Wq, np.float32)
    Wk = np.asarray(Wk, np.float32)
    Wv = np.asarray(Wv, np.float32)
    bq = np.asarray(bq, np.float32)
    bk = np.asarray(bk, np.float32)
    bv = np.asarray(bv, np.float32)

    wkT = Wk.T.copy()                                   # [j, f]
    # wv[jt][p=f%128][fkt][j%128] = Wv[fkt*128+p, jt*128+j]
    wv_tiled = np.ascontiguousarray(
        Wv.reshape(JC, 128, JC, 128).transpose(2, 1, 0, 3).astype(bft)
    )
    # onesbv[p][fkt][0]=1, [1]=bv[fkt*128+p]
    onesbv = np.empty((128, JC, 2), np.float32)
    onesbv[:, :, 0] = 1.0
    onesbv[:, :, 1] = bv.reshape(JC, 128).T
    onesbv = np.ascontiguousarray(onesbv.astype(bft))

    in_maps = []
    for c in range(N_CORES):
        pair_idx = next(i for i, g in enumerate(PAIRS) if c in g)
        b = pair_idx
        h = PAIRS[pair_idx].index(c)
        hb = h * EH
        perm = np.concatenate(
            [np.arange(hb, hb + EH), np.arange((1 - h) * EH, (1 - h) * EH + EH)]
        )
        xb = x[b]                                       # [S, E]
        x_perm = np.ascontiguousarray(xb[:, perm].astype(np.float16))
        # wkt[pc][p=j%128][jc][f] = wkT[perm[jc*128+p], pc*2*FC+f], fp16
        wkt_perm = np.ascontiguousarray(
            wkT[perm, :].reshape(JC, 128, NFC // 2, 2 * FC)
            .transpose(2, 1, 0, 3).astype(np.float16)
        )
        wq_h = (Wq[hb:hb + EH, :] * (sc * 64.0)).T      # [i, e'] scaled, x64
        wq_t = wq_h.reshape(JC, 128, EH)                # [ic, p, e']
        wqlo = np.ascontiguousarray(
            wq_t[:, :, 0:EH // 2].transpose(1, 0, 2).astype(np.float16))
        wqhi = np.ascontiguousarray(
            wq_t[:, :, EH // 2:EH].transpose(1, 0, 2).astype(np.float16))
        xsum = xb.sum(axis=0)                           # [E]
        c_vec = Wq[hb:hb + EH, :] @ xsum                # [EH]
        u_vec = Wk @ xsum + np.float32(S) * bk          # [E]
        bias_lhs = np.ascontiguousarray(
            np.stack([bq[hb:hb + EH] * (sc * 64.0),
                      c_vec * (sc * 64.0)]).astype(np.float16)
        )                                               # [2, EH]
        bias_rhs = np.ascontiguousarray(
            np.stack([u_vec, bk]).astype(np.float16)
        )                                               # [2, E]
        # xt[sb][p=j%128][jc][s] = x^T[jc*128+p, sb*SB+s]
        xt_t = np.ascontiguousarray(
            xb.T.reshape(JC, 128, NFC, SB).transpose(2, 1, 0, 3).astype(bft)
        )
        in_maps.append({
            "x_nat": x_perm,
            "wkt": wkt_perm,
            "wqlo": wqlo,
            "wqhi": wqhi,
            "bias_lhs": bias_lhs,
            "bias_rhs": bias_rhs,
            "wv": wv_tiled,
            "onesbv": onesbv,
            "xt": xt_t,
        })
    return in_maps


def run(in_maps, trace=False, **kwargs):
    nc = _get_nc()
    return run_bass_kernel_spmd(
        nc, in_maps, core_ids=list(range(N_CORES)), trace=trace, **kwargs
    )


def kernel(x, Wq, bq, Wk, bk, Wv, bv):
    in_maps = make_in_maps(x, Wq, bq, Wk, bk, Wv, bv)
    res = run(in_maps, trace=False)
    out = np.empty((B, E, S), dtype=np.float32)
    for c in range(N_CORES):
        pair_idx = next(i for i, g in enumerate(PAIRS) if c in g)
        b = pair_idx
        h = PAIRS[pair_idx].index(c)
        out[b, h * EH:(h + 1) * EH, :] = res.results[c]["outt"]
    return out
